# revision 2
# baseline (speedup 1.0000x reference)
"""EMA recurrence kernel for Trainium2 (8 NeuronCores, batch-parallel).

Computes c[b,t,d] = x[b,t,d] + decay * c[b,t-1,d]  (decay = sigmoid(decay_logit))
for x of shape (8, 4096, 2048) fp32, as a blocked scan:

  - T is split into chunks of L=127 rows. Within a chunk the scan is a
    triangular matmul: out[t,d] = sum_{s<=t} decay^(t-s) x[s,d].
  - The cross-chunk carry (c at the last row of the previous chunk) is folded
    into the same matmul as an extra contraction row whose weight column is
    decay^(t+1) — so each chunk is ONE matmul per 512-wide D tile.
  - Everything is bf16 on the wire (the correctness gate is 2e-2 rel err;
    bf16 end-to-end lands ~2e-3): the host casts x fp32->bf16, the kernel
    reads/writes bf16 HBM (halving DMA traffic vs fp32), matmuls run
    bf16 x bf16 -> fp32 PSUM, and output copies cast PSUM fp32 -> bf16.
    The host upcasts y back to fp32.
  - Layout: the carry input row lives at SBUF partition 0 (x rows at
    partitions 1..127), and the matmul's output columns are permuted so that
    PSUM partition 0 holds the chunk's LAST scan position (the next carry)
    and partitions 1..127 hold scan positions 0..126.  All compute-engine
    access patterns therefore start at partition 0 (the BIR verifier rejects
    engine APs starting at non-32-aligned partitions); only DMA (which has
    no partition-alignment restriction) touches rows 1..127.  Chunk 0 has no
    carry: its matmul reads a zeroed carry row.
  - Carry copies run on ScalarE straight from PSUM (so the PE chain does not
    wait on VectorE's output copies); output copies run on VectorE.
  - DMA queue split: in-DMAs ride the SWDGE ring (nc.gpsimd), out-DMAs ride
    the HWDGE SP ring (nc.sync).  Two independent rings means input prefetch
    and output drain stream concurrently with no head-of-line blocking
    between directions (the fp32 baseline had everything on one ring at
    ~140 GB/s effective; the rings each sustain ~184 GB/s).
  - Batch b is sharded across the 8 cores (one b per core).
"""

import os
import sys

os.environ.setdefault("MYCRO_LOCAL_CACHE", "1")
if "/opt/trn_rl_repo" not in sys.path:
    sys.path.insert(0, "/opt/trn_rl_repo")

from contextlib import ExitStack

import numpy as np
import ml_dtypes

B, T, D = 8, 4096, 2048
L = 127                 # x rows per main chunk (+1 carry row = K of 128)
NCHUNK = T // L         # 32 full chunks (ids 0..31)
TAIL = T - NCHUNK * L   # 32 trailing rows (chunk id 32)
DT = 512                # D tile width (one PSUM bank of fp32)
NT = D // DT            # 4 D tiles
GSZ = 2                 # chunks per SBUF tile group
N_CORES = 8
LTW = 128 + (TAIL + 1) + D  # weights + a zero row for chunk 0's carry

_compiled = {}


def _build_weights(decay_logit: np.ndarray):
    # Match the reference: decay = sigmoid(decay_logit) evaluated in fp32,
    # powers computed in fp64 from that fp32 value, rounded to bf16.
    logit = np.float64(np.asarray(decay_logit, dtype=np.float32))
    decay = np.float64(np.float32(1.0 / (1.0 + np.exp(-logit))))

    def lhs_t(rows, with_carry):
        # lhsT is [K, M]; out = lhsT.T @ rhs.
        # Output column m: m=0 is the carry-out (scan position rows-1),
        # m=1+t is scan position t.
        # Contraction p: with_carry -> p=0 is the carry row, p=1+s is x row s;
        # else p=s is x row s.
        pw = decay ** np.arange(rows + 1, dtype=np.float64)
        tri = np.zeros((rows, rows), np.float64)
        for s in range(rows):
            tri[s, s:] = pw[: rows - s]
        k = rows + 1 if with_carry else rows
        m = np.zeros((k, rows + 1), np.float64)
        if with_carry:
            m[0, 0] = pw[rows]          # carry -> carry-out
            m[1:, 0] = pw[rows - 1 :: -1]
            m[0, 1:] = pw[1:]           # carry -> position t
            m[1:, 1:] = tri
        else:
            m[:, 0] = pw[rows - 1 :: -1]
            m[:, 1:] = tri
        return m.astype(ml_dtypes.bfloat16)

    lt_main = lhs_t(L, with_carry=True)     # [128, 128]
    lt_tail = lhs_t(TAIL, with_carry=True)  # [33, 33]

    packed = np.zeros((128, LTW), ml_dtypes.bfloat16)
    packed[:, 0:128] = lt_main
    packed[: TAIL + 1, 128 : 128 + TAIL + 1] = lt_tail
    # columns 128+TAIL+1 .. end stay zero: the initial carry row for chunk 0
    return packed


def _build_program():
    import concourse.bacc as bacc
    import concourse.mybir as mybir
    from concourse.tile import TileContext

    f32 = mybir.dt.float32
    bf16 = mybir.dt.bfloat16
    nc = bacc.Bacc(trn_type="TRN2", target_bir_lowering=False, debug=False)

    x_d = nc.dram_tensor("x", [T, D], bf16, kind="ExternalInput")
    lt_d = nc.dram_tensor("lt_all", [128, LTW], bf16, kind="ExternalInput")
    y_d = nc.dram_tensor("y", [T, D], bf16, kind="ExternalOutput")

    # group g covers chunk ids GSZ*g .. min(GSZ*(g+1), 32)
    groups = []
    k = 0
    while k <= NCHUNK:  # ids 0..32
        ids = list(range(k, min(k + GSZ, NCHUNK + 1)))
        groups.append(ids)
        k += GSZ
    chunk_rows = [L] * NCHUNK + [TAIL]

    with TileContext(nc) as tc, ExitStack() as ctx:
        const = ctx.enter_context(tc.tile_pool(name="const", bufs=1))
        lt = const.tile([128, LTW], bf16, name="lt")
        nc.sync.dma_start(lt[:, :], lt_d[:, :])
        lt_main = lt[0:128, 0:128]
        lt_tail = lt[0 : TAIL + 1, 128 : 128 + TAIL + 1]
        zrow = lt[0:1, 128 + TAIL + 1 : 128 + TAIL + 1 + D]

        xin_pool = ctx.enter_context(tc.tile_pool(name="xin", bufs=6))
        yout_pool = ctx.enter_context(tc.tile_pool(name="yout", bufs=4))
        ps_pool = ctx.enter_context(tc.tile_pool(name="ps", bufs=8, space="PSUM"))

        xmap = {}  # chunk id -> (tile, col_base)
        ymap = {}

        def emit_in_dma(g):
            # per-chunk 2D dma_starts on the SWDGE (gpsimd) ring
            ids = groups[g]
            xt = xin_pool.tile([128, GSZ * D], bf16, name=f"xg{g}", tag="xg")
            for ci, i in enumerate(ids):
                rows = chunk_rows[i]
                nc.gpsimd.dma_start(
                    xt[1 : rows + 1, ci * D : ci * D + D],
                    x_d[i * L : i * L + rows, :],
                )
                xmap[i] = (xt, ci * D)

        def emit_out_dma(g):
            # out-DMAs on the HWDGE SP ring: independent of the in-DMA ring
            ids = groups[g]
            yt, _ = ymap[ids[0]]
            for ci, i in enumerate(ids):
                rows = chunk_rows[i]
                nc.sync.dma_start(
                    y_d[i * L : i * L + rows, :],
                    yt[1 : rows + 1, ci * D : ci * D + D],
                )

        def compute_chunk(k):
            rows = chunk_rows[k]
            lhsT = lt_tail if k == NCHUNK else lt_main
            xt, xcb = xmap[k]
            yt, ycb = ymap[k]
            m = rows + 1  # psum partitions (row 0 = carry-out)
            for j in range(NT):
                ps = ps_pool.tile([m, DT], f32, name=f"ps{k}_{j}", tag="ps")
                nc.tensor.matmul(
                    ps[:, :],
                    lhsT,
                    xt[0 : lhsT.shape[0], xcb + j * DT : xcb + (j + 1) * DT],
                    start=True,
                    stop=True,
                )
                if k + 1 <= NCHUNK:
                    nxt, ncb = xmap[k + 1]
                    # carry row for chunk k+1, on ScalarE straight from PSUM
                    nc.scalar.copy(
                        nxt[0:1, ncb + j * DT : ncb + (j + 1) * DT],
                        ps[0:1, :],
                    )
                nc.vector.tensor_copy(
                    yt[0:m, ycb + j * DT : ycb + (j + 1) * DT], ps[:, :]
                )

        # ---- emission order ----
        # In-DMAs are emitted a few groups early so the SWDGE ring stays fed;
        # out-DMAs live on their own ring so they can't block the input
        # stream regardless of order.
        emit_in_dma(0)
        nc.scalar.copy(xmap[0][0][0:1, 0:D], zrow)  # chunk 0 carry = 0
        emit_in_dma(1)
        emit_in_dma(2)

        for g in range(len(groups)):
            if g + 3 < len(groups):
                emit_in_dma(g + 3)
            if g >= 1:
                emit_out_dma(g - 1)
            yt = yout_pool.tile([128, GSZ * D], bf16, name=f"yg{g}", tag="yg")
            for ci, i in enumerate(groups[g]):
                ymap[i] = (yt, ci * D)
            for k in groups[g]:
                compute_chunk(k)
        emit_out_dma(len(groups) - 1)

    nc.finalize()
    return nc


def _get_program():
    if "nc" not in _compiled:
        _compiled["nc"] = _build_program()
    return _compiled["nc"]


def _install_profile_hook():
    """The container's `antenv` lacks `axon_hooks`, so NTFF profiling under
    axon degrades silently. Synthesize the module and install the ctypes hook
    from trn_agent_boot (same thing boot() would have done)."""
    if "antenv.axon_hooks" in sys.modules:
        return
    import types

    import antenv

    mod = types.ModuleType("antenv.axon_hooks")
    state = {"hook": None}
    mod.set_axon_ntff_profile_hook = lambda h: state.__setitem__("hook", h)
    mod.get_axon_ntff_profile_hook = lambda: state["hook"]
    sys.modules["antenv.axon_hooks"] = mod
    antenv.axon_hooks = mod

    from trn_agent_boot.trn_boot import _ntff_profile_via_ctypes

    mod.set_axon_ntff_profile_hook(
        _ntff_profile_via_ctypes("/opt/axon/libaxon_pjrt.so")
    )

    # no S3 in this container — keep artifacts local
    from concourse import bass_utils

    bass_utils.upload_artifacts = lambda tmpdir: tmpdir


def _run(x, decay_logit, trace=False):
    from concourse.bass_utils import run_bass_kernel_spmd

    if trace:
        _install_profile_hook()

    x = np.asarray(x, dtype=np.float32)
    assert x.shape == (B, T, D), x.shape
    x_bf = np.ascontiguousarray(x.astype(ml_dtypes.bfloat16))
    lt_all = _build_weights(decay_logit)

    nc = _get_program()
    in_maps = [
        {"x": np.ascontiguousarray(x_bf[b]), "lt_all": lt_all}
        for b in range(N_CORES)
    ]
    res = run_bass_kernel_spmd(
        nc,
        in_maps,
        core_ids=list(range(N_CORES)),
        trace=trace,
        trace_cores=[0] if trace else None,
    )
    y = np.stack(
        [np.asarray(res.results[b]["y"]).astype(np.float32) for b in range(N_CORES)],
        axis=0,
    )
    return y, res


def kernel(x, decay_logit):
    y, _ = _run(x, decay_logit, trace=False)
    return y


def kernel_traced(x, decay_logit):
    """Like kernel() but returns (y, BassKernelResults) with NTFF profile."""
    return _run(x, decay_logit, trace=True)


# revision 4
# speedup vs baseline: 3.6903x; 3.6903x over previous
"""EMA recurrence kernel for Trainium2 (8 NeuronCores, batch-parallel).

Computes c[b,t,d] = x[b,t,d] + decay * c[b,t-1,d]  (decay = sigmoid(decay_logit))
for x of shape (8, 4096, 2048) fp32, as a blocked scan in bf16 (gate is 2e-2
rel err; bf16 end-to-end lands ~4e-3).  Host casts x fp32->bf16 and upcasts y.

Blocked-scan structure (per core, batch row b):

  - T=4096 is split into 32 chunks of L=127 positions + a 32-row tail.
    Within a chunk the scan is a triangular matmul out[t] = sum_{s<=t}
    decay^(t-s) x[s] plus decay^(t+1) * carry, with the carry as an extra
    contraction row.
  - DMA shape rule (measured): ONLY [0:128]-partition transfers get the
    16-SDMA-engine descriptor spray (~294 GB/s read / ~241 GB/s write per
    core); anything else lands on ONE engine (~26 GB/s).  So every bulk
    transfer here is an exact 128-partition, 128-contiguous-DRAM-row op:
      in:  X_k[0:128]  <- x rows [k*127-1 .. k*127+126]   (ascending)
      out: y rows [k*127-1 .. k*127+126] <- Y_k[0:128]
    Partition 0 of X_k is the dead overlap row x[k*127-1]; the ScalarE carry
    copy overwrites it with the EMA carry before the matmuls run.  Output
    column 0 of the main matmul re-emits the carry-in verbatim, so the out-DMA
    writes y[k*127-1] twice (chunk k-1's position 126 and chunk k's column 0)
    with byte-identical values - benign.
  - The carry-out (position 126) would land at PSUM partition 127, which
    compute engines cannot address (32-alignment rule).  Instead a separate
    M=1 carry matmul (lhsT [128,1]) computes it straight into PSUM partition
    0 of a tiny [1,512] bank, and ScalarE copies it into X_{k+1}[0:1].  The
    serial cross-chunk chain is carry-mm -> ScalarE -> carry-mm (~1.2us per
    chunk); main matmuls and everything else hang off it with slack.
  - Main matmuls are bf16 x bf16 -> fp32 PSUM, N=512 (one PSUM bank), two
    banks per [128,1024] tile; VectorE drains each tile once (fp32->bf16).
  - Chunk 0 has no row -1: its in/out DMAs use the slow [1:128] shape, split
    into 4 column-ops so 4 SDMA engines share the work.
  - All bulk DMA rides the SWDGE (gpsimd) ring: ~241 GB/s measured with
    both directions interleaved at [0:128] shapes.
"""

import os
import sys

os.environ.setdefault("MYCRO_LOCAL_CACHE", "1")
if "/opt/trn_rl_repo" not in sys.path:
    sys.path.insert(0, "/opt/trn_rl_repo")

from contextlib import ExitStack

import numpy as np
import ml_dtypes

B, T, D = 8, 4096, 2048
L = 127                 # positions per main chunk
NCHUNK = T // L         # 32 main chunks (ids 0..31)
TAIL = T - NCHUNK * L   # 32 trailing positions (chunk id 32)
DT = 512                # matmul N (one PSUM bank of fp32)
NT = D // DT            # 4 D tiles
N_CORES = 8
# packed weights: lt_main [128,128] | lt_carry [128,1] | lt_tail [33,33] | zeros [*,D]
LTW = 128 + 1 + (TAIL + 1) + D

# out-DMA engine per chunk id (filled in _build_program)
OUT_ON_HWDGE = False  # flip to route out-DMAs to the HWDGE rings

_compiled = {}


def _build_weights(decay_logit: np.ndarray):
    logit = np.float64(np.asarray(decay_logit, dtype=np.float32))
    decay = np.float64(np.float32(1.0 / (1.0 + np.exp(-logit))))

    # main lhsT [128, 128]: contraction p=0 carry row, p=j x row (j-1).
    # out col m=0: carry-in passthrough; m=i (1..127): position i-1.
    lt_main = np.zeros((128, 128), np.float64)
    lt_main[0, 0] = 1.0
    for i in range(1, 128):
        lt_main[0, i] = decay ** i
        for j in range(1, i + 1):
            lt_main[j, i] = decay ** (i - j)

    # carry lhsT [128, 1]: out = position 126 (the carry-out)
    lt_carry = np.zeros((128, 1), np.float64)
    lt_carry[0, 0] = decay ** 127
    for j in range(1, 128):
        lt_carry[j, 0] = decay ** (127 - j)

    # tail lhsT [33, 33]: p=0 carry, p=j x row (j-1); m=0 passthrough,
    # m=i position i-1 (i=1..32)
    lt_tail = np.zeros((33, 33), np.float64)
    lt_tail[0, 0] = 1.0
    for i in range(1, 33):
        lt_tail[0, i] = decay ** i
        for j in range(1, i + 1):
            lt_tail[j, i] = decay ** (i - j)

    packed = np.zeros((128, LTW), ml_dtypes.bfloat16)
    packed[:, 0:128] = lt_main.astype(ml_dtypes.bfloat16)
    packed[:, 128:129] = lt_carry.astype(ml_dtypes.bfloat16)
    packed[: TAIL + 1, 129 : 129 + TAIL + 1] = lt_tail.astype(ml_dtypes.bfloat16)
    # columns 129+33 .. end stay zero: the initial carry row for chunk 0
    return packed


def _build_program():
    import concourse.bacc as bacc
    import concourse.mybir as mybir
    from concourse.tile import TileContext

    f32 = mybir.dt.float32
    bf16 = mybir.dt.bfloat16
    nc = bacc.Bacc(trn_type="TRN2", target_bir_lowering=False, debug=False)

    x_d = nc.dram_tensor("x", [T, D], bf16, kind="ExternalInput")
    lt_d = nc.dram_tensor("lt_all", [128, LTW], bf16, kind="ExternalInput")
    y_d = nc.dram_tensor("y", [T, D], bf16, kind="ExternalOutput")

    NCH = NCHUNK + 1  # 33 incl tail
    PF = 5            # in-DMA prefetch depth (chunks ahead)

    with TileContext(nc) as tc, ExitStack() as ctx:
        const = ctx.enter_context(tc.tile_pool(name="const", bufs=1))
        lt = const.tile([128, LTW], bf16, name="lt")
        nc.sync.dma_start(lt[:, :], lt_d[:, :])
        lt_main = lt[0:128, 0:128]
        lt_carry = lt[0:128, 128:129]
        lt_tail = lt[0 : TAIL + 1, 129 : 129 + TAIL + 1]
        zrow = lt[0:1, 129 + TAIL + 1 : 129 + TAIL + 1 + D]

        xin_pool = ctx.enter_context(tc.tile_pool(name="xin", bufs=PF + 2))
        yout_pool = ctx.enter_context(tc.tile_pool(name="yout", bufs=6))
        ps_pool = ctx.enter_context(tc.tile_pool(name="ps", bufs=3, space="PSUM"))
        psc_pool = ctx.enter_context(tc.tile_pool(name="psc", bufs=2, space="PSUM"))

        xmap = {}
        ymap = {}

        def emit_in_dma(k):
            xt = xin_pool.tile([128, D], bf16, name=f"x{k}", tag="xg")
            xmap[k] = xt
            if k == 0:
                # no row -1: slow [1:128] shape, split into 4 column ops
                for j in range(NT):
                    nc.gpsimd.dma_start(
                        xt[1 : L + 1, j * DT : (j + 1) * DT],
                        x_d[0:L, j * DT : (j + 1) * DT],
                    )
            elif k == NCHUNK:
                nc.gpsimd.dma_start(
                    xt[0 : TAIL + 1, :], x_d[T - TAIL - 1 : T, :]
                )
            else:
                nc.gpsimd.dma_start(
                    xt[0:128, :], x_d[k * L - 1 : k * L + L, :]
                )

        def emit_out_dma(k):
            # out-DMAs ride the two HWDGE rings (sync/scalar), which also get
            # the 16-engine spray for [0:128] shapes (~225 GB/s combined) and
            # run independently of the SWDGE in-stream.
            yt = ymap.pop(k)
            eng = nc.sync if k % 2 == 0 else nc.scalar
            if k == 0:
                # slow [1:128] shape: keep on SWDGE where 4 ops use 4 engines
                for j in range(NT):
                    nc.gpsimd.dma_start(
                        y_d[0:L, j * DT : (j + 1) * DT],
                        yt[1 : L + 1, j * DT : (j + 1) * DT],
                    )
            elif k == NCHUNK:
                eng.dma_start(
                    y_d[T - TAIL - 1 : T, :], yt[0 : TAIL + 1, :]
                )
            else:
                eng.dma_start(
                    y_d[k * L - 1 : k * L + L, :], yt[0:128, :]
                )

        def compute_chunk(k):
            tail = k == NCHUNK
            rows = TAIL if tail else L
            m = rows + 1
            xt = xmap[k]
            yt = yout_pool.tile([128, D], bf16, name=f"y{k}", tag="yg")
            ymap[k] = yt
            if not tail:
                # carry matmuls first: they drive the serial chain
                for j in range(NT):
                    psc = psc_pool.tile([1, DT], f32, name=f"pc{k}_{j}", tag="pc")
                    nc.tensor.matmul(
                        psc[:, :],
                        lt_carry,
                        xt[0:128, j * DT : (j + 1) * DT],
                        start=True,
                        stop=True,
                    )
                    nc.scalar.copy(
                        xmap[k + 1][0:1, j * DT : (j + 1) * DT], psc[0:1, :]
                    )
            lhsT = lt_tail if tail else lt_main
            for h in range(NT // 2):
                ps = ps_pool.tile([m, 2 * DT], f32, name=f"ps{k}_{h}", tag="ps")
                for jj in range(2):
                    j = 2 * h + jj
                    nc.tensor.matmul(
                        ps[:, jj * DT : (jj + 1) * DT],
                        lhsT,
                        xt[0 : lhsT.shape[0], j * DT : (j + 1) * DT],
                        start=True,
                        stop=True,
                    )
                nc.vector.tensor_copy(
                    yt[0:m, 2 * h * DT : 2 * (h + 1) * DT], ps[:, :]
                )

        # prologue: prefetch + zero-carry for chunk 0
        emit_in_dma(0)
        nc.scalar.copy(xmap[0][0:1, 0:D], zrow)
        for k in range(1, PF + 1):
            emit_in_dma(k)

        for k in range(NCH):
            if k + PF + 1 < NCH:
                emit_in_dma(k + PF + 1)
            if k >= 1:
                emit_out_dma(k - 1)
            compute_chunk(k)
        emit_out_dma(NCH - 1)

    nc.finalize()
    return nc


def _get_program():
    if "nc" not in _compiled:
        _compiled["nc"] = _build_program()
    return _compiled["nc"]


def _install_profile_hook():
    """The container's `antenv` lacks `axon_hooks`, so NTFF profiling under
    axon degrades silently. Synthesize the module and install the ctypes hook
    from trn_agent_boot (same thing boot() would have done)."""
    if "antenv.axon_hooks" in sys.modules:
        return
    import types

    import antenv

    mod = types.ModuleType("antenv.axon_hooks")
    state = {"hook": None}
    mod.set_axon_ntff_profile_hook = lambda h: state.__setitem__("hook", h)
    mod.get_axon_ntff_profile_hook = lambda: state["hook"]
    sys.modules["antenv.axon_hooks"] = mod
    antenv.axon_hooks = mod

    from trn_agent_boot.trn_boot import _ntff_profile_via_ctypes

    mod.set_axon_ntff_profile_hook(
        _ntff_profile_via_ctypes("/opt/axon/libaxon_pjrt.so")
    )

    # no S3 in this container — keep artifacts local
    from concourse import bass_utils

    bass_utils.upload_artifacts = lambda tmpdir: tmpdir


def _run(x, decay_logit, trace=False):
    from concourse.bass_utils import run_bass_kernel_spmd

    if trace:
        _install_profile_hook()

    x = np.asarray(x, dtype=np.float32)
    assert x.shape == (B, T, D), x.shape
    x_bf = np.ascontiguousarray(x.astype(ml_dtypes.bfloat16))
    lt_all = _build_weights(decay_logit)

    nc = _get_program()
    in_maps = [
        {"x": np.ascontiguousarray(x_bf[b]), "lt_all": lt_all}
        for b in range(N_CORES)
    ]
    res = run_bass_kernel_spmd(
        nc,
        in_maps,
        core_ids=list(range(N_CORES)),
        trace=trace,
        trace_cores=[0] if trace else None,
    )
    y = np.stack(
        [np.asarray(res.results[b]["y"]).astype(np.float32) for b in range(N_CORES)],
        axis=0,
    )
    return y, res


def kernel(x, decay_logit):
    y, _ = _run(x, decay_logit, trace=False)
    return y


def kernel_traced(x, decay_logit):
    """Like kernel() but returns (y, BassKernelResults) with NTFF profile."""
    return _run(x, decay_logit, trace=True)


# revision 5
# speedup vs baseline: 3.9086x; 1.0591x over previous
"""EMA recurrence kernel for Trainium2 (8 NeuronCores, batch-parallel).

Computes c[b,t,d] = x[b,t,d] + decay * c[b,t-1,d]  (decay = sigmoid(decay_logit))
for x of shape (8, 4096, 2048) fp32, as a blocked scan in bf16 (gate is 2e-2
rel err; bf16 end-to-end lands ~4e-3).  Host casts x fp32->bf16 and upcasts y.

Blocked-scan structure (per core, batch row b):

  - T=4096 is split into 32 chunks of L=127 positions + a 32-row tail.
    Within a chunk the scan is a triangular matmul out[t] = sum_{s<=t}
    decay^(t-s) x[s] plus decay^(t+1) * carry, with the carry as an extra
    contraction row.
  - DMA shape rule (measured): ONLY [0:128]-partition transfers get the
    16-SDMA-engine descriptor spray (~294 GB/s read / ~241 GB/s write per
    core); anything else lands on ONE engine (~26 GB/s).  So every bulk
    transfer here is an exact 128-partition, 128-contiguous-DRAM-row op:
      in:  X_k[0:128]  <- x rows [k*127-1 .. k*127+126]   (ascending)
      out: y rows [k*127-1 .. k*127+126] <- Y_k[0:128]
    Partition 0 of X_k is the dead overlap row x[k*127-1]; the ScalarE carry
    copy overwrites it with the EMA carry before the matmuls run.  Output
    column 0 of the main matmul re-emits the carry-in verbatim, so the out-DMA
    writes y[k*127-1] twice (chunk k-1's position 126 and chunk k's column 0)
    with byte-identical values - benign.
  - The carry-out (position 126) would land at PSUM partition 127, which
    compute engines cannot address (32-alignment rule).  Instead a separate
    M=1 carry matmul (lhsT [128,1]) computes it straight into PSUM partition
    0 of a tiny [1,512] bank, and ScalarE copies it into X_{k+1}[0:1].  The
    serial cross-chunk chain is carry-mm -> ScalarE -> carry-mm (~1.2us per
    chunk); main matmuls and everything else hang off it with slack.
  - Main matmuls are bf16 x bf16 -> fp32 PSUM, N=512 (one PSUM bank), two
    banks per [128,1024] tile; VectorE drains each tile once (fp32->bf16).
  - Chunk 0 has no row -1: its in/out DMAs use the slow [1:128] shape, split
    into 4 column-ops so 4 SDMA engines share the work.
  - All bulk DMA rides the SWDGE (gpsimd) ring: ~241 GB/s measured with
    both directions interleaved at [0:128] shapes.
"""

import os
import sys

os.environ.setdefault("MYCRO_LOCAL_CACHE", "1")
if "/opt/trn_rl_repo" not in sys.path:
    sys.path.insert(0, "/opt/trn_rl_repo")

from contextlib import ExitStack

import numpy as np
import ml_dtypes

B, T, D = 8, 4096, 2048
L = 127                 # positions per main chunk
NCHUNK = T // L         # 32 main chunks (ids 0..31)
TAIL = T - NCHUNK * L   # 32 trailing positions (chunk id 32)
DT = 512                # matmul N (one PSUM bank of fp32)
NT = D // DT            # 4 D tiles
N_CORES = 8
# packed weights: lt_main [128,128] | lt_carry [128,1] | lt_tail [33,33] | zeros [*,D]
LTW = 128 + 1 + (TAIL + 1) + D

# out-DMA engine per chunk id (filled in _build_program)
OUT_ON_HWDGE = False  # flip to route out-DMAs to the HWDGE rings

_compiled = {}


def _build_weights(decay_logit: np.ndarray):
    logit = np.float64(np.asarray(decay_logit, dtype=np.float32))
    decay = np.float64(np.float32(1.0 / (1.0 + np.exp(-logit))))

    # main lhsT [128, 128]: contraction p=0 carry row, p=j x row (j-1).
    # out col m=0: carry-in passthrough; m=i (1..127): position i-1.
    lt_main = np.zeros((128, 128), np.float64)
    lt_main[0, 0] = 1.0
    for i in range(1, 128):
        lt_main[0, i] = decay ** i
        for j in range(1, i + 1):
            lt_main[j, i] = decay ** (i - j)

    # carry lhsT [128, 1]: out = position 126 (the carry-out)
    lt_carry = np.zeros((128, 1), np.float64)
    lt_carry[0, 0] = decay ** 127
    for j in range(1, 128):
        lt_carry[j, 0] = decay ** (127 - j)

    # tail lhsT [33, 33]: p=0 carry, p=j x row (j-1); m=0 passthrough,
    # m=i position i-1 (i=1..32)
    lt_tail = np.zeros((33, 33), np.float64)
    lt_tail[0, 0] = 1.0
    for i in range(1, 33):
        lt_tail[0, i] = decay ** i
        for j in range(1, i + 1):
            lt_tail[j, i] = decay ** (i - j)

    packed = np.zeros((128, LTW), ml_dtypes.bfloat16)
    packed[:, 0:128] = lt_main.astype(ml_dtypes.bfloat16)
    packed[:, 128:129] = lt_carry.astype(ml_dtypes.bfloat16)
    packed[: TAIL + 1, 129 : 129 + TAIL + 1] = lt_tail.astype(ml_dtypes.bfloat16)
    # columns 129+33 .. end stay zero: the initial carry row for chunk 0
    return packed


def _build_program():
    import concourse.bacc as bacc
    import concourse.mybir as mybir
    from concourse.tile import TileContext

    f32 = mybir.dt.float32
    bf16 = mybir.dt.bfloat16
    nc = bacc.Bacc(trn_type="TRN2", target_bir_lowering=False, debug=False)

    x_d = nc.dram_tensor("x", [T, D], bf16, kind="ExternalInput")
    lt_d = nc.dram_tensor("lt_all", [128, LTW], bf16, kind="ExternalInput")
    y_d = nc.dram_tensor("y", [T, D], bf16, kind="ExternalOutput")

    NCH = NCHUNK + 1  # 33 incl tail
    PF = 7            # in-DMA prefetch depth (chunks ahead)

    with TileContext(nc) as tc, ExitStack() as ctx:
        const = ctx.enter_context(tc.tile_pool(name="const", bufs=1))
        lt = const.tile([128, LTW], bf16, name="lt")
        nc.sync.dma_start(lt[:, :], lt_d[:, :])
        lt_main = lt[0:128, 0:128]
        lt_carry = lt[0:128, 128:129]
        lt_tail = lt[0 : TAIL + 1, 129 : 129 + TAIL + 1]
        zrow = lt[0:1, 129 + TAIL + 1 : 129 + TAIL + 1 + D]

        xin_pool = ctx.enter_context(tc.tile_pool(name="xin", bufs=PF + 2))
        yout_pool = ctx.enter_context(tc.tile_pool(name="yout", bufs=8))
        ps_pool = ctx.enter_context(tc.tile_pool(name="ps", bufs=3, space="PSUM"))
        psc_pool = ctx.enter_context(tc.tile_pool(name="psc", bufs=2, space="PSUM"))

        xmap = {}
        ymap = {}

        def emit_in_dma(k):
            xt = xin_pool.tile([128, D], bf16, name=f"x{k}", tag="xg")
            xmap[k] = xt
            if k == 0:
                # no row -1: slow [1:128] shape, split into 4 column ops
                for j in range(NT):
                    nc.gpsimd.dma_start(
                        xt[1 : L + 1, j * DT : (j + 1) * DT],
                        x_d[0:L, j * DT : (j + 1) * DT],
                    )
            elif k == NCHUNK:
                nc.gpsimd.dma_start(
                    xt[0 : TAIL + 1, :], x_d[T - TAIL - 1 : T, :]
                )
            else:
                nc.gpsimd.dma_start(
                    xt[0:128, :], x_d[k * L - 1 : k * L + L, :]
                )

        def emit_out_dma(k):
            # out-DMAs ride the two HWDGE rings (sync/scalar), which also get
            # the 16-engine spray for [0:128] shapes (~225 GB/s combined) and
            # run independently of the SWDGE in-stream.
            # ALL on nc.sync: the SP engine issues nothing else, so a
            # blocked out-DMA issue can't head-of-line block the ScalarE
            # carry chain (nc.scalar is an HWDGE engine too - measured 7.4us
            # chain stalls when out-DMAs shared ACT's queue).
            yt = ymap.pop(k)
            eng = nc.sync
            if k == 0:
                # slow [1:128] shape: keep on SWDGE where 4 ops use 4 engines
                for j in range(NT):
                    nc.gpsimd.dma_start(
                        y_d[0:L, j * DT : (j + 1) * DT],
                        yt[1 : L + 1, j * DT : (j + 1) * DT],
                    )
            elif k == NCHUNK:
                eng.dma_start(
                    y_d[T - TAIL - 1 : T, :], yt[0 : TAIL + 1, :]
                )
            else:
                eng.dma_start(
                    y_d[k * L - 1 : k * L + L, :], yt[0:128, :]
                )

        def compute_chunk(k):
            tail = k == NCHUNK
            rows = TAIL if tail else L
            m = rows + 1
            xt = xmap[k]
            yt = yout_pool.tile([128, D], bf16, name=f"y{k}", tag="yg")
            ymap[k] = yt
            if not tail:
                # carry matmuls first: they drive the serial chain
                for j in range(NT):
                    psc = psc_pool.tile([1, DT], f32, name=f"pc{k}_{j}", tag="pc")
                    nc.tensor.matmul(
                        psc[:, :],
                        lt_carry,
                        xt[0:128, j * DT : (j + 1) * DT],
                        start=True,
                        stop=True,
                    )
                    nc.scalar.copy(
                        xmap[k + 1][0:1, j * DT : (j + 1) * DT], psc[0:1, :]
                    )
            lhsT = lt_tail if tail else lt_main
            for h in range(NT // 2):
                ps = ps_pool.tile([m, 2 * DT], f32, name=f"ps{k}_{h}", tag="ps")
                for jj in range(2):
                    j = 2 * h + jj
                    nc.tensor.matmul(
                        ps[:, jj * DT : (jj + 1) * DT],
                        lhsT,
                        xt[0 : lhsT.shape[0], j * DT : (j + 1) * DT],
                        start=True,
                        stop=True,
                    )
                nc.vector.tensor_copy(
                    yt[0:m, 2 * h * DT : 2 * (h + 1) * DT], ps[:, :]
                )

        # prologue: prefetch + zero-carry for chunk 0
        emit_in_dma(0)
        nc.scalar.copy(xmap[0][0:1, 0:D], zrow)
        for k in range(1, PF + 1):
            emit_in_dma(k)

        for k in range(NCH):
            if k + PF + 1 < NCH:
                emit_in_dma(k + PF + 1)
            if k >= 1:
                emit_out_dma(k - 1)
            compute_chunk(k)
        emit_out_dma(NCH - 1)

    nc.finalize()
    return nc


def _get_program():
    if "nc" not in _compiled:
        _compiled["nc"] = _build_program()
    return _compiled["nc"]


def _install_profile_hook():
    """The container's `antenv` lacks `axon_hooks`, so NTFF profiling under
    axon degrades silently. Synthesize the module and install the ctypes hook
    from trn_agent_boot (same thing boot() would have done)."""
    if "antenv.axon_hooks" in sys.modules:
        return
    import types

    import antenv

    mod = types.ModuleType("antenv.axon_hooks")
    state = {"hook": None}
    mod.set_axon_ntff_profile_hook = lambda h: state.__setitem__("hook", h)
    mod.get_axon_ntff_profile_hook = lambda: state["hook"]
    sys.modules["antenv.axon_hooks"] = mod
    antenv.axon_hooks = mod

    from trn_agent_boot.trn_boot import _ntff_profile_via_ctypes

    mod.set_axon_ntff_profile_hook(
        _ntff_profile_via_ctypes("/opt/axon/libaxon_pjrt.so")
    )

    # no S3 in this container — keep artifacts local
    from concourse import bass_utils

    bass_utils.upload_artifacts = lambda tmpdir: tmpdir


def _run(x, decay_logit, trace=False):
    from concourse.bass_utils import run_bass_kernel_spmd

    if trace:
        _install_profile_hook()

    x = np.asarray(x, dtype=np.float32)
    assert x.shape == (B, T, D), x.shape
    x_bf = np.ascontiguousarray(x.astype(ml_dtypes.bfloat16))
    lt_all = _build_weights(decay_logit)

    nc = _get_program()
    in_maps = [
        {"x": np.ascontiguousarray(x_bf[b]), "lt_all": lt_all}
        for b in range(N_CORES)
    ]
    res = run_bass_kernel_spmd(
        nc,
        in_maps,
        core_ids=list(range(N_CORES)),
        trace=trace,
        trace_cores=[0] if trace else None,
    )
    y = np.stack(
        [np.asarray(res.results[b]["y"]).astype(np.float32) for b in range(N_CORES)],
        axis=0,
    )
    return y, res


def kernel(x, decay_logit):
    y, _ = _run(x, decay_logit, trace=False)
    return y


def kernel_traced(x, decay_logit):
    """Like kernel() but returns (y, BassKernelResults) with NTFF profile."""
    return _run(x, decay_logit, trace=True)
